# revision 21
# baseline (speedup 1.0000x reference)
"""Trainium2 Bass kernel for ProbSparse multi-head attention (L_Q = 1).

Math: with L_Q=1 the reference's top-k/sampling machinery is identity
(top-1 of a length-1 axis is index 0 and the scatter overwrites the whole
context), so the computation reduces to single-query attention:

  out[b] = concat_h( softmax((q Wq)_h . (k Wk)_h^T / 8) @ (v Wv)_h ) @ Wo + bo

Low-rank restructuring (L_Q = 1): fold the projections through the single
query / attention vector so the big k/v tensors are consumed by exactly one
streaming pass each:

  r[b,h,:]   = Wk_h @ (qh[b,h]/8)          (host, f64)
  scoresT[s,h] = sum_c k[b,s,c] r[b,h,c]   (device, PE)
  attnT      = exp(scoresT)                 (device, ACT; unnormalized)
  w[b,h,:]   = sum_s attnT[s,h] v[b,s,:]    (device, PE)
  Z[b,h]     = sum_s attnT[s,h]             (device, PE; ones column)
  out[b]     = concat_h((w/Z) Wv_h + bv_h) @ Wo + bo   (host)

v3 (this file): everything except the two streaming passes moves to the
host.  k is staged as fp8 e3m4 (4-bit mantissa; measured end-to-end error
0.0066 vs the 2e-2 gate), v as bf16.  Scores use kt as the matmul
STATIONARY operand so PSUM receives scoresT [128s, 16h] directly: no PE
transposes, full-width ACT exp, and attnT feeds the V-phase matmuls as
stationary with vt moving.  Per-core DMA is 24.3 MB vs the v2 design's
38 MB, and the single fused stream keeps DMA busy continuously while PE
gaps stay under the HAM re-throttle window.

Sharding: data-parallel over batch, 2 batches per core, 8 cores.
"""

import sys

sys.path.insert(0, "/opt/trn_rl_repo")

import numpy as np
import ml_dtypes

import concourse.bass as bass
import concourse.mybir as mybir
from bass_rust import add_dep_helper
import concourse.tile_sem_assignment as _tsa
from concourse.tile import TileContext
from concourse import bass_utils


# ---- framework patch (from v2): the kernel-tail drain aggregates one
# semaphore wait per active proc, exceeding the 1-wait DRAIN encoding.
# Split the waits across a chain of single-wait drains.
from concourse.tile import TileContext as _TC
from concourse.vector_clock import ScopedClock as _SC

def _split_drain_and_barrier(self, tick_clock, wait_clock):
    drain_inst = self.nc.sync.drain()
    wait_clock.add_sem_waits(drain_inst.ins, _SC({None: tick_clock.global_clock}))
    si = drain_inst.ins.sync_info
    if si is not None and si.on_wait and len(si.on_wait) > 1:
        waits = list(si.on_wait)
        si.on_wait = waits[:1]
        for w in waits[1:]:
            d2 = self.nc.sync.drain()
            s2 = d2.ins.sync_info
            if s2 is None:
                d2.ins.sync_info = type(si)(on_wait=[w], on_update=[])
            else:
                s2.on_wait = [w]
    self.nc.all_engine_barrier()
    assert self.sems is not None
    popped = self.nc._tile_sem_poison_stack.pop()
    assert popped is self._sem_poison
    self.nc.clear_and_free_semaphores(list(self.sems.allocated().values()))
    self.nc.all_engine_barrier()

_TC._drain_and_barrier = _split_drain_and_barrier

B, H, DH, HID, LK = 16, 16, 64, 1024, 4096
NCORES = 8
BL = B // NCORES            # batches per core
NCH = HID // 128            # 8 hidden chunks of 128
NT = LK // 1024             # 4 stream tiles of 1024 seq positions per batch
NU = BL * NT                # 8 stream units per core
N_WARM = 8                  # PE warmup matmuls (N=512, ~0.43us each cold)

f32 = mybir.dt.float32
bf16 = mybir.dt.bfloat16
fp8 = mybir.dt.float8e3
FT = mybir.ActivationFunctionType
AX = mybir.AxisListType

bf16_np = ml_dtypes.bfloat16
fp8_np = ml_dtypes.float8_e3m4


def build_nc():
    # one DMA-completion semaphore lane per DGE type: consumers then never
    # accumulate multi-lane DMA waits (several instruction structs allow
    # only 1-2 sync waits).
    _tsa.NUM_HWDGE_SEMS = 1
    _tsa.NUM_SWDGE_GLOBAL_SEMS = 1

    nc = bass.Bass("TRN2")

    kt_d = nc.dram_tensor("kt_loc", [NU, 128, NCH, 1024], fp8, kind="ExternalInput")
    v_d = nc.dram_tensor("v_loc", [NU, 128, NCH, 1024], bf16, kind="ExternalInput")
    blob_d = nc.dram_tensor("blob", [128, 260], bf16, kind="ExternalInput")
    w_d = nc.dram_tensor("w_loc", [BL, H, HID], f32, kind="ExternalOutput")
    z_d = nc.dram_tensor("z_loc", [H, BL], f32, kind="ExternalOutput")

    with TileContext(nc) as tc:
        with tc.tile_pool(name="main", bufs=1) as mp, \
             tc.tile_pool(name="ps", bufs=1, space="PSUM") as pp:

            # ---- constants ----
            blob = mp.tile([128, 260], bf16, tag="blob")
            nc.scalar.dma_start(out=blob, in_=blob_d[:, :])
            r_sb = blob[:, 0:256].rearrange("p (ch bl h) -> p ch bl h", ch=NCH, bl=BL)

            # ---- long-lived accumulators ----
            psum_w = [pp.tile([H, HID], f32, tag=f"w{bl}", name=f"w{bl}")
                      for bl in range(BL)]
            # per-(unit, half) softmax normalizer partials via ACT accum_out
            Zs = mp.tile([H, 2 * NU], f32, tag="Zs")

            # ---- PE warmup: keep the PE busy during the initial DMA fill so
            # the HAM clock gate reaches 8/8 before real work arrives.  The
            # warmup groups use psum_w[0]'s bank and close before the real
            # accumulation group starts. ----
            warm_sb = mp.tile([128, H], bf16, tag="warm")
            nc.vector.memset(warm_sb, 0.0)
            warm_mv = mp.tile([128, 512], bf16, tag="warmmv")
            nc.vector.memset(warm_mv, 0.0)
            for i in range(N_WARM):
                nc.tensor.matmul(psum_w[0][:, 0:512], warm_sb, warm_mv,
                                 start=True, stop=True)

            # ---- fused stream: 8 units of (kt 1MB fp8, vt 2MB bf16).
            # The V phase runs one unit behind scores (software pipeline) so
            # the DVE transposes of unit u overlap with scores of unit u+1
            # and the PE never waits on DVE. ----
            def emit_v_phase(uu, attnT_u, vt_u):
                bl_u, t_u = divmod(uu, NT)
                nc.tensor.ldweights(vt_u[:, 0, 0:1])
                for j in range(8):
                    first = (t_u == 0 and j == 0)
                    last = (t_u == NT - 1 and j == 7)
                    for hf in range(2):
                        nc.tensor.matmul(
                            psum_w[bl_u][:, hf * 512:(hf + 1) * 512],
                            attnT_u[:, j, 0:H],
                            vt_u[:, j, hf * 512:(hf + 1) * 512],
                            start=first, stop=last)

            prev = None
            smm_last = {}
            for u in range(NU):
                bl, t = divmod(u, NT)
                # Dual-ring DMA: kt on the SP HWDGE ring, vt on the ACT ring,
                # so each ring's ~1.7us completion/setup overhead hides behind
                # the other ring's transfer.  The SP nop teaches the SP clock
                # the PE sem value so the kt DMA's buffer-WAR wait elides; on
                # the ACT ring the exps do the same for vt.
                kt = mp.tile([128, NCH, 1024], fp8, tag="kt", bufs=4)
                if u >= 4:
                    n = nc.sync.nop()
                    add_dep_helper(n.ins, smm_last[u - 4].ins, reason="sp-clock")
                nc.sync.dma_start(out=kt, in_=kt_d[u])
                vt = mp.tile([128, NCH, 1024], bf16, tag="vt", bufs=4)
                nc.scalar.dma_start(out=vt, in_=v_d[u])

                # scores[h, s] = sum_c r[c, h] kt[c, s]: r stationary, kt
                # moving in wide N=512 matmuls.  The leading ldweights absorbs
                # the kt-DMA wait.
                nc.tensor.ldweights(kt[:, 0, 0:1])
                attn = mp.tile([32, 8, 4, 32], bf16, tag="attn", bufs=NU)
                attnT = mp.tile([128, 8, 32], bf16, tag="attnT", bufs=NU)
                attn_flat = attn.rearrange("p j b c -> p (j b c)")
                for h in range(2):
                    pss = pp.tile([H, 512], f32, tag="s", bufs=4)
                    for cj in range(NCH):
                        smm = nc.tensor.matmul(
                            pss,
                            r_sb[:, cj, bl, :],
                            kt[:, cj, h * 512:(h + 1) * 512],
                            start=(cj == 0), stop=(cj == NCH - 1))
                    smm_last[u] = smm
                    nc.scalar.activation(
                        attn_flat[0:H, h * 512:(h + 1) * 512], pss, FT.Exp,
                        accum_out=Zs[:, 2 * u + h:2 * u + h + 1])
                    # DVE assembles attnT via batched 32x32 block transposes:
                    # one instruction per 32-partition destination group moves
                    # 4 blocks (rows 16:32 of attn are junk; the junk columns
                    # of attnT are never read).
                    for b in range(4):
                        nc.vector.transpose(
                            attnT[b * 32:(b + 1) * 32, 4 * h:4 * h + 4, :],
                            attn[0:32, 4 * h:4 * h + 4, b, :])

                if prev is not None:
                    emit_v_phase(u - 1, *prev)
                prev = (attnT, vt)

            emit_v_phase(NU - 1, *prev)

            # ---- drain: PSUM -> SBUF -> DRAM ----
            w_sb = mp.tile([H, BL, HID], f32, tag="wsb")
            z_sb = mp.tile([H, BL], f32, tag="zsb")
            for bl in range(BL):
                nc.vector.tensor_copy(w_sb[:, bl, :], psum_w[bl])
            for bl in range(BL):
                nc.vector.reduce_sum(z_sb[:, bl:bl + 1],
                                     Zs[:, bl * 2 * NT:(bl + 1) * 2 * NT],
                                     axis=AX.X)
            # ratchets: teach the ACT clock the final DVE ticks (the bl=1
            # drain copies are the last DVE work) so the output DMAs carry
            # only their ring wait; explicit deps pin the order.
            scratch = mp.tile([1, 2], f32, tag="scratch")
            ratA = nc.scalar.copy(scratch[0:1, 0:1], w_sb[0:1, 1, 0:1])
            ratB = nc.scalar.copy(scratch[0:1, 1:2], z_sb[0:1, 1:2])
            add_dep_helper(ratB.ins, ratA.ins, reason="act-order")
            for bl in range(BL):
                d = nc.scalar.dma_start(out=w_d[bl], in_=w_sb[:, bl, :])
                add_dep_helper(d.ins, ratB.ins, reason="act-order")
            d = nc.scalar.dma_start(out=z_d[:, :], in_=z_sb)
            add_dep_helper(d.ins, ratB.ins, reason="act-order")

    # Post-pass: strip redundant semaphore waits so no instruction exceeds
    # its struct's 1-wait budget.  Every engine sequencer issues in FIFO
    # order, so (a) a wait on the engine's own completion semaphore is
    # trivially satisfied, and (b) a wait on sem>=v is redundant when an
    # earlier instruction on the same engine already waited sem>=v'>=v.
    eng_sem = {
        mybir.EngineType.Activation: "Activation",
        mybir.EngineType.PE: "PE",
        mybir.EngineType.DVE: "DVE",
        mybir.EngineType.SP: "SP",
        mybir.EngineType.Pool: "Pool",
    }
    known = {}
    for fn in nc.m.functions:
        for b in fn.blocks:
            for ins in b.instructions:
                si = ins.sync_info
                if si is None or not si.on_wait:
                    continue
                eng = ins.engine
                kn = known.setdefault(eng, {})
                if len(si.on_wait) > 1:
                    own = eng_sem.get(eng)
                    keep = []
                    for w in si.on_wait:
                        if own is not None and w.ant_name.startswith(own + "_"):
                            continue
                        if kn.get(w.ant_name, -1) >= w.wait_value:
                            continue
                        keep.append(w)
                    if not keep:
                        keep = list(si.on_wait)[:1]
                    si.on_wait = keep
                for w in si.on_wait:
                    if w.wait_value > kn.get(w.ant_name, -1):
                        kn[w.ant_name] = w.wait_value

    return nc


def make_in_maps(q, k, v, Wq, bq, Wv, bv, Wo, bo, Wk):
    scale = DH ** -0.5
    q64, Wq64, bq64, Wk64 = (np.asarray(a, np.float64) for a in (q, Wq, bq, Wk))
    qh = q64 @ Wq64 + bq64                      # [B, HID]
    r = np.zeros((B, H, HID))
    for h in range(H):
        blk = slice(h * DH, (h + 1) * DH)
        r[:, h, :] = qh[:, blk] @ Wk64[:, blk].T * scale

    in_maps = []
    for c in range(NCORES):
        sl = slice(BL * c, BL * (c + 1))
        # kt: [u, p, ch, s] = k[bl, t*1024+s, ch*128+p], u = bl*NT + t
        kk = np.ascontiguousarray(k[sl].transpose(0, 2, 1))        # [BL, HID, LK]
        kt = kk.reshape(BL, NCH, 128, NT, 1024).transpose(0, 3, 2, 1, 4) \
            .reshape(NU, 128, NCH, 1024)
        # vt: [u, p, blk, c] = v[bl, t*1024 + blk*128 + p, c]
        vt = v[sl].reshape(BL, NT, 8, 128, HID).transpose(0, 1, 3, 2, 4) \
            .reshape(NU, 128, 8, 1024)
        blob = np.zeros((128, 260), dtype=np.float64)
        rc = r[sl]                                                  # [BL, H, HID]
        # blob[p, (ch, bl, h)] = rc[bl, h, ch*128+p]
        blob[:, 0:256] = rc.transpose(2, 0, 1).reshape(NCH, 128, BL, H) \
            .transpose(1, 0, 2, 3).reshape(128, 256)
        blob[:, 256] = 1.0
        in_maps.append({
            "kt_loc": np.ascontiguousarray(kt).astype(fp8_np),
            "v_loc": np.ascontiguousarray(vt).astype(bf16_np),
            "blob": blob.astype(bf16_np),
        })
    return in_maps


def finish_host(results, Wv, bv, Wo, bo):
    """Apply the Wv / Wo tail on the host: out = concat_h((w/Z) Wv_h + bv) Wo + bo."""
    Wv64, bv64 = np.asarray(Wv, np.float64), np.asarray(bv, np.float64)
    Wo64, bo64 = np.asarray(Wo, np.float64), np.asarray(bo, np.float64)
    out = np.zeros((B, HID))
    for c, res in enumerate(results):
        w = np.asarray(res["w_loc"], np.float64)       # [BL, H, HID]
        z = np.asarray(res["z_loc"], np.float64)       # [H, BL]
        for bl in range(BL):
            x = np.zeros(HID)
            xs = []
            for h in range(H):
                blk = slice(h * DH, (h + 1) * DH)
                xs.append(w[bl, h] / z[h, bl] @ Wv64[:, blk] + bv64[blk])
            x = np.concatenate(xs)
            out[c * BL + bl] = x @ Wo64 + bo64
    return out.astype(np.float32)


_nc_cache = None


def kernel(q, k, v, index_sample, Wq, bq, Wk, bk, Wv, bv, Wo, bo):
    global _nc_cache
    q, k, v = (np.asarray(a, np.float32) for a in (q, k, v))
    # bk provably cancels in the softmax (constant shift per (b, h) row).
    if _nc_cache is None:
        _nc_cache = build_nc()
    nc = _nc_cache
    in_maps = make_in_maps(q, k, v, Wq, bq, Wv, bv, Wo, bo, Wk)
    res = bass_utils.run_bass_kernel_spmd(nc, in_maps, core_ids=list(range(NCORES)))
    return finish_host(res.results, Wv, bv, Wo, bo)


# revision 29
# speedup vs baseline: 1.1079x; 1.1079x over previous
"""Trainium2 Bass kernel for ProbSparse multi-head attention (L_Q = 1).

Math: with L_Q=1 the reference's top-k/sampling machinery is identity
(top-1 of a length-1 axis is index 0 and the scatter overwrites the whole
context), so the computation reduces to single-query attention:

  out[b] = concat_h( softmax((q Wq)_h . (k Wk)_h^T / 8) @ (v Wv)_h ) @ Wo + bo

Low-rank restructuring (L_Q = 1): fold the projections through the single
query / attention vector so the big k/v tensors are consumed by exactly one
streaming pass each:

  r[b,h,:]   = Wk_h @ (qh[b,h]/8)          (host, f64)
  scoresT[s,h] = sum_c k[b,s,c] r[b,h,c]   (device, PE)
  attnT      = exp(scoresT)                 (device, ACT; unnormalized)
  w[b,h,:]   = sum_s attnT[s,h] v[b,s,:]    (device, PE)
  Z[b,h]     = sum_s attnT[s,h]             (device, PE; ones column)
  out[b]     = concat_h((w/Z) Wv_h + bv_h) @ Wo + bo   (host)

v3 (this file): everything except the two streaming passes moves to the
host.  k is staged as fp8 e3m4 (4-bit mantissa; measured end-to-end error
0.0066 vs the 2e-2 gate), v as bf16.  Scores use kt as the matmul
STATIONARY operand so PSUM receives scoresT [128s, 16h] directly: no PE
transposes, full-width ACT exp, and attnT feeds the V-phase matmuls as
stationary with vt moving.  Per-core DMA is 24.3 MB vs the v2 design's
38 MB, and the single fused stream keeps DMA busy continuously while PE
gaps stay under the HAM re-throttle window.

Sharding: data-parallel over batch, 2 batches per core, 8 cores.
"""

import sys

sys.path.insert(0, "/opt/trn_rl_repo")

import numpy as np
import ml_dtypes

import concourse.bass as bass
import concourse.mybir as mybir
from bass_rust import add_dep_helper
import concourse.tile_sem_assignment as _tsa
from concourse.tile import TileContext
from concourse import bass_utils


# ---- framework patch (from v2): the kernel-tail drain aggregates one
# semaphore wait per active proc, exceeding the 1-wait DRAIN encoding.
# Split the waits across a chain of single-wait drains.
from concourse.tile import TileContext as _TC
from concourse.vector_clock import ScopedClock as _SC

def _split_drain_and_barrier(self, tick_clock, wait_clock):
    drain_inst = self.nc.sync.drain()
    wait_clock.add_sem_waits(drain_inst.ins, _SC({None: tick_clock.global_clock}))
    si = drain_inst.ins.sync_info
    if si is not None and si.on_wait and len(si.on_wait) > 1:
        waits = list(si.on_wait)
        si.on_wait = waits[:1]
        for w in waits[1:]:
            d2 = self.nc.sync.drain()
            s2 = d2.ins.sync_info
            if s2 is None:
                d2.ins.sync_info = type(si)(on_wait=[w], on_update=[])
            else:
                s2.on_wait = [w]
    self.nc.all_engine_barrier()
    assert self.sems is not None
    popped = self.nc._tile_sem_poison_stack.pop()
    assert popped is self._sem_poison
    self.nc.clear_and_free_semaphores(list(self.sems.allocated().values()))
    self.nc.all_engine_barrier()

_TC._drain_and_barrier = _split_drain_and_barrier

B, H, DH, HID, LK = 16, 16, 64, 1024, 4096
NCORES = 8
BL = B // NCORES            # batches per core
NCH = HID // 128            # 8 hidden chunks of 128
NT = LK // 1024             # 4 stream tiles of 1024 seq positions per batch
NU = BL * NT                # 8 stream units per core
N_WARM = 8                  # PE warmup matmuls (N=512, ~0.43us each cold)

f32 = mybir.dt.float32
bf16 = mybir.dt.bfloat16
fp8 = mybir.dt.float8e3
FT = mybir.ActivationFunctionType
AX = mybir.AxisListType

bf16_np = ml_dtypes.bfloat16
fp8_np = ml_dtypes.float8_e3m4


def build_nc():
    nc = bass.Bass("TRN2")

    kt_d = nc.dram_tensor("kt_loc", [NU, 128, NCH, 1024], fp8, kind="ExternalInput")
    v_d = nc.dram_tensor("v_loc", [NU, 128, NCH, 1024], bf16, kind="ExternalInput")
    blob_d = nc.dram_tensor("blob", [128, 260], bf16, kind="ExternalInput")
    w_d = nc.dram_tensor("w_loc", [BL, H, HID], f32, kind="ExternalOutput")
    z_d = nc.dram_tensor("z_loc", [H, BL], f32, kind="ExternalOutput")

    with TileContext(nc) as tc:
        with tc.tile_pool(name="main", bufs=1) as mp, \
             tc.tile_pool(name="ps", bufs=1, space="PSUM") as pp:

            # ---- constants ----
            blob = mp.tile([128, 260], bf16, tag="blob")
            nc.scalar.dma_start(out=blob, in_=blob_d[:, :])
            r_sb = blob[:, 0:256].rearrange("p (ch bl h) -> p ch bl h", ch=NCH, bl=BL)

            # ---- long-lived accumulators ----
            psum_w = [pp.tile([H, HID], f32, tag=f"w{bl}", name=f"w{bl}")
                      for bl in range(BL)]
            # per-(unit, half) softmax normalizer partials via ACT accum_out
            Zs = mp.tile([H, 2 * NU], f32, tag="Zs")

            # ---- PE warmup: keep the PE busy during the initial DMA fill so
            # the HAM clock gate reaches 8/8 before real work arrives.  The
            # warmup groups use psum_w[0]'s bank and close before the real
            # accumulation group starts. ----
            warm_sb = mp.tile([128, H], bf16, tag="warm")
            nc.vector.memset(warm_sb, 0.0)
            warm_mv = mp.tile([128, 512], bf16, tag="warmmv")
            nc.vector.memset(warm_mv, 0.0)
            for i in range(N_WARM):
                nc.tensor.matmul(psum_w[0][:, 0:512], warm_sb, warm_mv,
                                 start=True, stop=True)

            # ---- fused stream: 8 units of (kt 1MB fp8, vt 2MB bf16).
            # The V phase runs one unit behind scores (software pipeline) so
            # the DVE transposes of unit u overlap with scores of unit u+1
            # and the PE never waits on DVE. ----
            def emit_v_phase(uu, attnT_u, vt_u):
                bl_u, t_u = divmod(uu, NT)
                nc.tensor.ldweights(vt_u[:, 0, 0:1])
                for j in range(8):
                    first = (t_u == 0 and j == 0)
                    last = (t_u == NT - 1 and j == 7)
                    for hf in range(2):
                        nc.tensor.matmul(
                            psum_w[bl_u][:, hf * 512:(hf + 1) * 512],
                            attnT_u[:, j, 0:H],
                            vt_u[:, j, hf * 512:(hf + 1) * 512],
                            start=first, stop=last)

            # All kt transfers queue upfront on the SP HWDGE ring (bufs=NU:
            # no buffer reuse, so no WAR waits), and most vt transfers queue
            # upfront on the ACT ring.  Both rings are saturated from t=0 and
            # the SDMA engines interleave the two streams at packet
            # granularity — no sequencer gating between transfers.
            NVB = 6
            lscr = mp.tile([1, 4], f32, tag="lscr")
            kts, vts = [], []
            for u in range(NU):
                kt = mp.tile([128, NCH, 1024], fp8, tag="kt", bufs=NU,
                             name=f"kt{u}")
                nc.sync.dma_start(out=kt, in_=kt_d[u])
                kts.append(kt)
            for u in range(NVB):
                vt = mp.tile([128, NCH, 1024], bf16, tag="vt", bufs=NVB,
                             name=f"vt{u}")
                nc.scalar.dma_start(out=vt, in_=v_d[u])
                vts.append(vt)

            prev = None
            for u in range(NU):
                bl, t = divmod(u, NT)
                kt = kts[u]
                vt = vts[u]

                # scores[h, s] = sum_c r[c, h] kt[c, s]: r stationary, kt
                # moving in wide N=512 matmuls.  The leading ldweights absorbs
                # the kt-DMA wait.
                nc.tensor.ldweights(kt[:, 0, 0:1])
                attn = mp.tile([32, 8, 4, 32], bf16, tag="attn", bufs=NU)
                attnT = mp.tile([128, 8, 32], bf16, tag="attnT", bufs=NU)
                attn_flat = attn.rearrange("p j b c -> p (j b c)")
                for h in range(2):
                    pss = pp.tile([H, 512], f32, tag="s", bufs=4)
                    for cj in range(NCH):
                        nc.tensor.matmul(
                            pss,
                            r_sb[:, cj, bl, :],
                            kt[:, cj, h * 512:(h + 1) * 512],
                            start=(cj == 0), stop=(cj == NCH - 1))
                    ei = nc.scalar.activation(
                        attn_flat[0:H, h * 512:(h + 1) * 512], pss, FT.Exp,
                        accum_out=Zs[:, 2 * u + h:2 * u + h + 1])
                    # DVE assembles attnT via batched 32x32 block transposes:
                    # one instruction per 32-partition destination group moves
                    # 4 blocks (rows 16:32 of attn are junk; the junk columns
                    # of attnT are never read).
                    for b in range(4):
                        nc.vector.transpose(
                            attnT[b * 32:(b + 1) * 32, 4 * h:4 * h + 4, :],
                            attn[0:32, 4 * h:4 * h + 4, b, :])

                if prev is not None:
                    emit_v_phase(u - 1, *prev)
                prev = (attnT, vt)

                # late vt DMAs: issued after this unit's exps.  The dep chain
                # exp -> ratchet-copy -> dma teaches the ACT clock the PE sem
                # (buffer WAR vs V(lu-NVB)) and the lane sem of the buffer
                # predecessor's transfer (WAW), so the dma keeps only its own
                # ring-slot wait.
                lu = u + 4
                if NVB <= lu < NU:
                    rv = nc.scalar.copy(lscr[0:1, lu - NVB:lu - NVB + 1],
                                        vts[lu - NVB][0:1, 0, 0:1])
                    add_dep_helper(rv.ins, ei.ins, reason="act-order")
                    lvt = mp.tile([128, NCH, 1024], bf16, tag="vt", bufs=NVB,
                                  name=f"vt{lu}")
                    d = nc.scalar.dma_start(out=lvt, in_=v_d[lu])
                    add_dep_helper(d.ins, rv.ins, reason="act-order")
                    vts.append(lvt)

            emit_v_phase(NU - 1, *prev)

            # ---- drain: PSUM -> SBUF -> DRAM ----
            w_sb = mp.tile([H, BL, HID], f32, tag="wsb")
            z_sb = mp.tile([H, BL], f32, tag="zsb")
            for bl in range(BL):
                nc.vector.tensor_copy(w_sb[:, bl, :], psum_w[bl])
            for bl in range(BL):
                nc.vector.reduce_sum(z_sb[:, bl:bl + 1],
                                     Zs[:, bl * 2 * NT:(bl + 1) * 2 * NT],
                                     axis=AX.X)
            # ratchets: teach the ACT clock the final DVE ticks (the bl=1
            # drain copies are the last DVE work) so the output DMAs carry
            # only their ring wait; explicit deps pin the order.
            scratch = mp.tile([1, 2], f32, tag="scratch")
            ratA = nc.scalar.copy(scratch[0:1, 0:1], w_sb[0:1, 1, 0:1])
            ratB = nc.scalar.copy(scratch[0:1, 1:2], z_sb[0:1, 1:2])
            add_dep_helper(ratB.ins, ratA.ins, reason="act-order")
            for bl in range(BL):
                d = nc.scalar.dma_start(out=w_d[bl], in_=w_sb[:, bl, :])
                add_dep_helper(d.ins, ratB.ins, reason="act-order")
            d = nc.scalar.dma_start(out=z_d[:, :], in_=z_sb)
            add_dep_helper(d.ins, ratB.ins, reason="act-order")

    # Post-pass: strip redundant semaphore waits so no instruction exceeds
    # its struct's 1-wait budget.  Every engine sequencer issues in FIFO
    # order, so (a) a wait on the engine's own completion semaphore is
    # trivially satisfied, and (b) a wait on sem>=v is redundant when an
    # earlier instruction on the same engine already waited sem>=v'>=v.
    eng_sem = {
        mybir.EngineType.Activation: "Activation",
        mybir.EngineType.PE: "PE",
        mybir.EngineType.DVE: "DVE",
        mybir.EngineType.SP: "SP",
        mybir.EngineType.Pool: "Pool",
    }
    known = {}
    for fn in nc.m.functions:
        for b in fn.blocks:
            for ins in b.instructions:
                si = ins.sync_info
                if si is None or not si.on_wait:
                    continue
                eng = ins.engine
                kn = known.setdefault(eng, {})
                if len(si.on_wait) > 1:
                    own = eng_sem.get(eng)
                    keep = []
                    for w in si.on_wait:
                        if own is not None and w.ant_name.startswith(own + "_"):
                            continue
                        if kn.get(w.ant_name, -1) >= w.wait_value:
                            continue
                        keep.append(w)
                    if not keep:
                        keep = list(si.on_wait)[:1]
                    si.on_wait = keep
                for w in si.on_wait:
                    if w.wait_value > kn.get(w.ant_name, -1):
                        kn[w.ant_name] = w.wait_value

    return nc


def make_in_maps(q, k, v, Wq, bq, Wv, bv, Wo, bo, Wk):
    scale = DH ** -0.5
    q64, Wq64, bq64, Wk64 = (np.asarray(a, np.float64) for a in (q, Wq, bq, Wk))
    qh = q64 @ Wq64 + bq64                      # [B, HID]
    r = np.zeros((B, H, HID))
    for h in range(H):
        blk = slice(h * DH, (h + 1) * DH)
        r[:, h, :] = qh[:, blk] @ Wk64[:, blk].T * scale

    in_maps = []
    for c in range(NCORES):
        sl = slice(BL * c, BL * (c + 1))
        # kt: [u, p, ch, s] = k[bl, t*1024+s, ch*128+p], u = bl*NT + t
        kk = np.ascontiguousarray(k[sl].transpose(0, 2, 1))        # [BL, HID, LK]
        kt = kk.reshape(BL, NCH, 128, NT, 1024).transpose(0, 3, 2, 1, 4) \
            .reshape(NU, 128, NCH, 1024)
        # vt: [u, p, blk, c] = v[bl, t*1024 + blk*128 + p, c]
        vt = v[sl].reshape(BL, NT, 8, 128, HID).transpose(0, 1, 3, 2, 4) \
            .reshape(NU, 128, 8, 1024)
        blob = np.zeros((128, 260), dtype=np.float64)
        rc = r[sl]                                                  # [BL, H, HID]
        # blob[p, (ch, bl, h)] = rc[bl, h, ch*128+p]
        blob[:, 0:256] = rc.transpose(2, 0, 1).reshape(NCH, 128, BL, H) \
            .transpose(1, 0, 2, 3).reshape(128, 256)
        blob[:, 256] = 1.0
        in_maps.append({
            "kt_loc": np.ascontiguousarray(kt).astype(fp8_np),
            "v_loc": np.ascontiguousarray(vt).astype(bf16_np),
            "blob": blob.astype(bf16_np),
        })
    return in_maps


def finish_host(results, Wv, bv, Wo, bo):
    """Apply the Wv / Wo tail on the host: out = concat_h((w/Z) Wv_h + bv) Wo + bo."""
    Wv64, bv64 = np.asarray(Wv, np.float64), np.asarray(bv, np.float64)
    Wo64, bo64 = np.asarray(Wo, np.float64), np.asarray(bo, np.float64)
    out = np.zeros((B, HID))
    for c, res in enumerate(results):
        w = np.asarray(res["w_loc"], np.float64)       # [BL, H, HID]
        z = np.asarray(res["z_loc"], np.float64)       # [H, BL]
        for bl in range(BL):
            x = np.zeros(HID)
            xs = []
            for h in range(H):
                blk = slice(h * DH, (h + 1) * DH)
                xs.append(w[bl, h] / z[h, bl] @ Wv64[:, blk] + bv64[blk])
            x = np.concatenate(xs)
            out[c * BL + bl] = x @ Wo64 + bo64
    return out.astype(np.float32)


_nc_cache = None


def kernel(q, k, v, index_sample, Wq, bq, Wk, bk, Wv, bv, Wo, bo):
    global _nc_cache
    q, k, v = (np.asarray(a, np.float32) for a in (q, k, v))
    # bk provably cancels in the softmax (constant shift per (b, h) row).
    if _nc_cache is None:
        _nc_cache = build_nc()
    nc = _nc_cache
    in_maps = make_in_maps(q, k, v, Wq, bq, Wv, bv, Wo, bo, Wk)
    res = bass_utils.run_bass_kernel_spmd(nc, in_maps, core_ids=list(range(NCORES)))
    return finish_host(res.results, Wv, bv, Wo, bo)


# revision 35
# speedup vs baseline: 1.3908x; 1.2553x over previous
"""Trainium2 Bass kernel for ProbSparse multi-head attention (L_Q = 1).

Math: with L_Q=1 the reference's top-k/sampling machinery is identity
(top-1 of a length-1 axis is index 0 and the scatter overwrites the whole
context), so the computation reduces to single-query attention:

  out[b] = concat_h( softmax((q Wq)_h . (k Wk)_h^T / 8) @ (v Wv)_h ) @ Wo + bo

Low-rank restructuring (L_Q = 1): fold the projections through the single
query / attention vector so the big k/v tensors are consumed by exactly one
streaming pass each:

  r[b,h,:]   = Wk_h @ (qh[b,h]/8)          (host, f64)
  scoresT[s,h] = sum_c k[b,s,c] r[b,h,c]   (device, PE)
  attnT      = exp(scoresT)                 (device, ACT; unnormalized)
  w[b,h,:]   = sum_s attnT[s,h] v[b,s,:]    (device, PE)
  Z[b,h]     = sum_s attnT[s,h]             (device, PE; ones column)
  out[b]     = concat_h((w/Z) Wv_h + bv_h) @ Wo + bo   (host)

v3 (this file): everything except the two streaming passes moves to the
host.  k is staged as fp8 e3m4 (4-bit mantissa; measured end-to-end error
0.0066 vs the 2e-2 gate), v as bf16.  Scores use kt as the matmul
STATIONARY operand so PSUM receives scoresT [128s, 16h] directly: no PE
transposes, full-width ACT exp, and attnT feeds the V-phase matmuls as
stationary with vt moving.  Per-core DMA is 24.3 MB vs the v2 design's
38 MB, and the single fused stream keeps DMA busy continuously while PE
gaps stay under the HAM re-throttle window.

Sharding: data-parallel over batch, 2 batches per core, 8 cores.
"""

import sys

sys.path.insert(0, "/opt/trn_rl_repo")

import numpy as np
import ml_dtypes

import concourse.bass as bass
import concourse.mybir as mybir
from bass_rust import add_dep_helper
import concourse.tile_sem_assignment as _tsa
from concourse.tile import TileContext
from concourse import bass_utils


# ---- framework patch (from v2): the kernel-tail drain aggregates one
# semaphore wait per active proc, exceeding the 1-wait DRAIN encoding.
# Split the waits across a chain of single-wait drains.
from concourse.tile import TileContext as _TC
from concourse.vector_clock import ScopedClock as _SC

def _split_drain_and_barrier(self, tick_clock, wait_clock):
    drain_inst = self.nc.sync.drain()
    wait_clock.add_sem_waits(drain_inst.ins, _SC({None: tick_clock.global_clock}))
    si = drain_inst.ins.sync_info
    if si is not None and si.on_wait and len(si.on_wait) > 1:
        waits = list(si.on_wait)
        si.on_wait = waits[:1]
        for w in waits[1:]:
            d2 = self.nc.sync.drain()
            s2 = d2.ins.sync_info
            if s2 is None:
                d2.ins.sync_info = type(si)(on_wait=[w], on_update=[])
            else:
                s2.on_wait = [w]
    self.nc.all_engine_barrier()
    assert self.sems is not None
    popped = self.nc._tile_sem_poison_stack.pop()
    assert popped is self._sem_poison
    self.nc.clear_and_free_semaphores(list(self.sems.allocated().values()))
    self.nc.all_engine_barrier()

_TC._drain_and_barrier = _split_drain_and_barrier

B, H, DH, HID, LK = 16, 16, 64, 1024, 4096
NCORES = 8
BL = B // NCORES            # batches per core
NCH = HID // 128            # 8 hidden chunks of 128
NT = LK // 1024             # 4 stream tiles of 1024 seq positions per batch
NU = BL * NT                # 8 stream units per core
N_WARM = 8                  # PE warmup matmuls (N=512, ~0.43us each cold)

f32 = mybir.dt.float32
bf16 = mybir.dt.bfloat16
fp8 = mybir.dt.float8e3
FT = mybir.ActivationFunctionType
AX = mybir.AxisListType

bf16_np = ml_dtypes.bfloat16
fp8_np = ml_dtypes.float8_e3m4


def build_nc():
    nc = bass.Bass("TRN2")

    kt_d = nc.dram_tensor("kt_loc", [NU, 128, NCH, 1024], fp8, kind="ExternalInput")
    v_d = nc.dram_tensor("v_loc", [NU, 128, NCH, 1024], bf16, kind="ExternalInput")
    blob_d = nc.dram_tensor("blob", [128, 260], bf16, kind="ExternalInput")
    w_d = nc.dram_tensor("w_loc", [BL, H, HID], bf16, kind="ExternalOutput")
    z_d = nc.dram_tensor("z_loc", [H, BL], f32, kind="ExternalOutput")

    with TileContext(nc) as tc:
        with tc.tile_pool(name="main", bufs=1) as mp, \
             tc.tile_pool(name="ps", bufs=1, space="PSUM") as pp:

            # ---- constants ----
            blob = mp.tile([128, 260], bf16, tag="blob")
            nc.scalar.dma_start(out=blob, in_=blob_d[:, :])
            r_sb = blob[:, 0:256].rearrange("p (ch bl h) -> p ch bl h", ch=NCH, bl=BL)

            # ---- long-lived accumulators ----
            psum_w = [pp.tile([H, HID], f32, tag=f"w{bl}", name=f"w{bl}")
                      for bl in range(BL)]
            # per-(unit, half) softmax normalizer partials via ACT accum_out
            Zs = mp.tile([H, 2 * NU], f32, tag="Zs")

            # ---- PE warmup: keep the PE busy during the initial DMA fill so
            # the HAM clock gate reaches 8/8 before real work arrives.  The
            # warmup groups use psum_w[0]'s bank and close before the real
            # accumulation group starts. ----
            warm_sb = mp.tile([128, H], bf16, tag="warm")
            nc.vector.memset(warm_sb, 0.0)
            warm_mv = mp.tile([128, 512], bf16, tag="warmmv")
            nc.vector.memset(warm_mv, 0.0)
            for i in range(N_WARM):
                nc.tensor.matmul(psum_w[0][:, 0:512], warm_sb, warm_mv,
                                 start=True, stop=True)

            # ---- fused stream: 8 units of (kt 1MB fp8, vt 2MB bf16).
            # The V phase runs one unit behind scores (software pipeline) so
            # the DVE transposes of unit u overlap with scores of unit u+1
            # and the PE never waits on DVE. ----
            def emit_v_phase(uu, attnT_u, vt_u):
                bl_u, t_u = divmod(uu, NT)
                nc.tensor.ldweights(vt_u[:, 0, 0:1])
                for j in range(8):
                    first = (t_u == 0 and j == 0)
                    last = (t_u == NT - 1 and j == 7)
                    for hf in range(2):
                        nc.tensor.matmul(
                            psum_w[bl_u][:, hf * 512:(hf + 1) * 512],
                            attnT_u[:, j, 0:H],
                            vt_u[:, j, hf * 512:(hf + 1) * 512],
                            start=first, stop=last)

            # All transfers queue upfront with no buffer reuse (no WAR
            # waits): kt on the SP HWDGE ring, vt via GpSimd SWDGE.  An HWDGE
            # dma_start blocks its issuing queue until the transfer completes,
            # so the streams live on queues with no compute work; ACT stays
            # free for the exps.  The SDMA engines round-robin the two queues
            # per descriptor (vt descriptors are 2x kt's bytes), splitting
            # bandwidth 2:1 vt:kt — the exact consumption ratio, so kt(u) and
            # vt(u) arrive just in time at a ~8.8us unit cadence.
            kts, vts = [], []
            for u in range(NU):
                kt = mp.tile([128, NCH, 1024], fp8, tag="kt", bufs=NU,
                             name=f"kt{u}")
                nc.sync.dma_start(out=kt, in_=kt_d[u])
                kts.append(kt)
            for u in range(NU):
                vt = mp.tile([128, NCH, 1024], bf16, tag="vt", bufs=NU,
                             name=f"vt{u}")
                nc.gpsimd.dma_start(out=vt, in_=v_d[u])
                vts.append(vt)

            scratch2 = mp.tile([1, NU], f32, tag="scratch2")
            attnTs = []
            prev = None
            for u in range(NU):
                bl, t = divmod(u, NT)
                kt = kts[u]
                vt = vts[u]

                # scores[h, s] = sum_c r[c, h] kt[c, s]: r stationary, kt
                # moving in wide N=512 matmuls.  The leading ldweights absorbs
                # the kt-DMA wait.
                nc.tensor.ldweights(kt[:, 0, 0:1])
                attn = mp.tile([32, 8, 4, 32], bf16, tag="attn", bufs=2)
                attnT = mp.tile([128, 8, 32], bf16, tag="attnT", bufs=NU)
                attn_flat = attn.rearrange("p j b c -> p (j b c)")
                # attn bufs=2: the buffer WAR vs the DVE transposes of unit
                # u-2 is taught to the ACT clock by a ratchet copy reading
                # attnT(u-1).
                rT = None
                if u >= 2:
                    rT = nc.scalar.copy(scratch2[0:1, u - 2:u - 1],
                                        attnTs[u - 1][0:1, 0, 0:1])
                for h in range(2):
                    pss = pp.tile([H, 512], f32, tag="s", bufs=4)
                    for cj in range(NCH):
                        nc.tensor.matmul(
                            pss,
                            r_sb[:, cj, bl, :],
                            kt[:, cj, h * 512:(h + 1) * 512],
                            start=(cj == 0), stop=(cj == NCH - 1))
                    ei = nc.scalar.activation(
                        attn_flat[0:H, h * 512:(h + 1) * 512], pss, FT.Exp,
                        accum_out=Zs[:, 2 * u + h:2 * u + h + 1])
                    if rT is not None and h == 0:
                        add_dep_helper(ei.ins, rT.ins, reason="act-order")
                    # DVE assembles attnT via batched 32x32 block transposes:
                    # one instruction per 32-partition destination group moves
                    # 4 blocks (rows 16:32 of attn are junk; the junk columns
                    # of attnT are never read).
                    for b in range(4):
                        nc.vector.transpose(
                            attnT[b * 32:(b + 1) * 32, 4 * h:4 * h + 4, :],
                            attn[0:32, 4 * h:4 * h + 4, b, :])

                attnTs.append(attnT)
                if prev is not None:
                    emit_v_phase(u - 1, *prev)
                prev = (attnT, vt)

            emit_v_phase(NU - 1, *prev)

            # ---- drain: PSUM -> SBUF -> DRAM ----
            w_sb = mp.tile([H, BL, HID], bf16, tag="wsb")
            z_sb = mp.tile([H, BL], f32, tag="zsb")
            for bl in range(BL):
                nc.vector.tensor_copy(w_sb[:, bl, :], psum_w[bl])
            for bl in range(BL):
                nc.vector.reduce_sum(z_sb[:, bl:bl + 1],
                                     Zs[:, bl * 2 * NT:(bl + 1) * 2 * NT],
                                     axis=AX.X)
            # ratchets: teach the ACT clock the final DVE ticks (the bl=1
            # drain copies are the last DVE work) so the output DMAs carry
            # only their ring wait; explicit deps pin the order.
            scratch = mp.tile([1, 2], f32, tag="scratch")
            ratA = nc.scalar.copy(scratch[0:1, 0:1], w_sb[0:1, 1, 0:1])
            ratB = nc.scalar.copy(scratch[0:1, 1:2], z_sb[0:1, 1:2])
            add_dep_helper(ratB.ins, ratA.ins, reason="act-order")
            for bl in range(BL):
                d = nc.scalar.dma_start(out=w_d[bl], in_=w_sb[:, bl, :])
                add_dep_helper(d.ins, ratB.ins, reason="act-order")
            d = nc.scalar.dma_start(out=z_d[:, :], in_=z_sb)
            add_dep_helper(d.ins, ratB.ins, reason="act-order")

    # Post-pass: strip redundant semaphore waits so no instruction exceeds
    # its struct's 1-wait budget.  Every engine sequencer issues in FIFO
    # order, so (a) a wait on the engine's own completion semaphore is
    # trivially satisfied, and (b) a wait on sem>=v is redundant when an
    # earlier instruction on the same engine already waited sem>=v'>=v.
    eng_sem = {
        mybir.EngineType.Activation: "Activation",
        mybir.EngineType.PE: "PE",
        mybir.EngineType.DVE: "DVE",
        mybir.EngineType.SP: "SP",
        mybir.EngineType.Pool: "Pool",
    }
    known = {}
    for fn in nc.m.functions:
        for b in fn.blocks:
            for ins in b.instructions:
                si = ins.sync_info
                if si is None or not si.on_wait:
                    continue
                eng = ins.engine
                kn = known.setdefault(eng, {})
                if len(si.on_wait) > 1:
                    own = eng_sem.get(eng)
                    keep = []
                    for w in si.on_wait:
                        if own is not None and w.ant_name.startswith(own + "_"):
                            continue
                        if kn.get(w.ant_name, -1) >= w.wait_value:
                            continue
                        keep.append(w)
                    if not keep:
                        keep = list(si.on_wait)[:1]
                    si.on_wait = keep
                for w in si.on_wait:
                    if w.wait_value > kn.get(w.ant_name, -1):
                        kn[w.ant_name] = w.wait_value

    return nc


def make_in_maps(q, k, v, Wq, bq, Wv, bv, Wo, bo, Wk):
    scale = DH ** -0.5
    q64, Wq64, bq64, Wk64 = (np.asarray(a, np.float64) for a in (q, Wq, bq, Wk))
    qh = q64 @ Wq64 + bq64                      # [B, HID]
    r = np.zeros((B, H, HID))
    for h in range(H):
        blk = slice(h * DH, (h + 1) * DH)
        r[:, h, :] = qh[:, blk] @ Wk64[:, blk].T * scale

    in_maps = []
    for c in range(NCORES):
        sl = slice(BL * c, BL * (c + 1))
        # kt: [u, p, ch, s] = k[bl, t*1024+s, ch*128+p], u = bl*NT + t
        kk = np.ascontiguousarray(k[sl].transpose(0, 2, 1))        # [BL, HID, LK]
        kt = kk.reshape(BL, NCH, 128, NT, 1024).transpose(0, 3, 2, 1, 4) \
            .reshape(NU, 128, NCH, 1024)
        # vt: [u, p, blk, c] = v[bl, t*1024 + blk*128 + p, c]
        vt = v[sl].reshape(BL, NT, 8, 128, HID).transpose(0, 1, 3, 2, 4) \
            .reshape(NU, 128, 8, 1024)
        blob = np.zeros((128, 260), dtype=np.float64)
        rc = r[sl]                                                  # [BL, H, HID]
        # blob[p, (ch, bl, h)] = rc[bl, h, ch*128+p]
        blob[:, 0:256] = rc.transpose(2, 0, 1).reshape(NCH, 128, BL, H) \
            .transpose(1, 0, 2, 3).reshape(128, 256)
        blob[:, 256] = 1.0
        in_maps.append({
            "kt_loc": np.ascontiguousarray(kt).astype(fp8_np),
            "v_loc": np.ascontiguousarray(vt).astype(bf16_np),
            "blob": blob.astype(bf16_np),
        })
    return in_maps


def finish_host(results, Wv, bv, Wo, bo):
    """Apply the Wv / Wo tail on the host: out = concat_h((w/Z) Wv_h + bv) Wo + bo."""
    Wv64, bv64 = np.asarray(Wv, np.float64), np.asarray(bv, np.float64)
    Wo64, bo64 = np.asarray(Wo, np.float64), np.asarray(bo, np.float64)
    out = np.zeros((B, HID))
    for c, res in enumerate(results):
        w = np.asarray(res["w_loc"], np.float64)       # [BL, H, HID]
        z = np.asarray(res["z_loc"], np.float64)       # [H, BL]
        for bl in range(BL):
            x = np.zeros(HID)
            xs = []
            for h in range(H):
                blk = slice(h * DH, (h + 1) * DH)
                xs.append(w[bl, h] / z[h, bl] @ Wv64[:, blk] + bv64[blk])
            x = np.concatenate(xs)
            out[c * BL + bl] = x @ Wo64 + bo64
    return out.astype(np.float32)


_nc_cache = None


def kernel(q, k, v, index_sample, Wq, bq, Wk, bk, Wv, bv, Wo, bo):
    global _nc_cache
    q, k, v = (np.asarray(a, np.float32) for a in (q, k, v))
    # bk provably cancels in the softmax (constant shift per (b, h) row).
    if _nc_cache is None:
        _nc_cache = build_nc()
    nc = _nc_cache
    in_maps = make_in_maps(q, k, v, Wq, bq, Wv, bv, Wo, bo, Wk)
    res = bass_utils.run_bass_kernel_spmd(nc, in_maps, core_ids=list(range(NCORES)))
    return finish_host(res.results, Wv, bv, Wo, bo)


# revision 40
# speedup vs baseline: 1.7529x; 1.2603x over previous
"""Trainium2 Bass kernel for ProbSparse multi-head attention (L_Q = 1).

Math: with L_Q=1 the reference's top-k/sampling machinery is identity
(top-1 of a length-1 axis is index 0 and the scatter overwrites the whole
context), so the computation reduces to single-query attention:

  out[b] = concat_h( softmax((q Wq)_h . (k Wk)_h^T / 8) @ (v Wv)_h ) @ Wo + bo

Low-rank restructuring (L_Q = 1): fold the projections through the single
query / attention vector so the big k/v tensors are consumed by exactly one
streaming pass each:

  r[b,h,:]   = Wk_h @ (qh[b,h]/8)          (host, f64)
  scoresT[s,h] = sum_c k[b,s,c] r[b,h,c]   (device, PE)
  attnT      = exp(scoresT)                 (device, ACT; unnormalized)
  w[b,h,:]   = sum_s attnT[s,h] v[b,s,:]    (device, PE)
  Z[b,h]     = sum_s attnT[s,h]             (device, PE; ones column)
  out[b]     = concat_h((w/Z) Wv_h + bv_h) @ Wo + bo   (host)

v3 (this file): everything except the two streaming passes moves to the
host.  k is staged as fp8 e3m4 (4-bit mantissa; measured end-to-end error
0.0066 vs the 2e-2 gate), v as bf16.  Scores use kt as the matmul
STATIONARY operand so PSUM receives scoresT [128s, 16h] directly: no PE
transposes, full-width ACT exp, and attnT feeds the V-phase matmuls as
stationary with vt moving.  Per-core DMA is 24.3 MB vs the v2 design's
38 MB, and the single fused stream keeps DMA busy continuously while PE
gaps stay under the HAM re-throttle window.

Sharding: data-parallel over batch, 2 batches per core, 8 cores.
"""

import sys

sys.path.insert(0, "/opt/trn_rl_repo")

import numpy as np
import ml_dtypes

import concourse.bass as bass
import concourse.mybir as mybir
from bass_rust import add_dep_helper
import concourse.tile_sem_assignment as _tsa
from concourse.tile import TileContext
from concourse import bass_utils


# ---- framework patch (from v2): the kernel-tail drain aggregates one
# semaphore wait per active proc, exceeding the 1-wait DRAIN encoding.
# Split the waits across a chain of single-wait drains.
from concourse.tile import TileContext as _TC
from concourse.vector_clock import ScopedClock as _SC

def _split_drain_and_barrier(self, tick_clock, wait_clock):
    drain_inst = self.nc.sync.drain()
    wait_clock.add_sem_waits(drain_inst.ins, _SC({None: tick_clock.global_clock}))
    si = drain_inst.ins.sync_info
    if si is not None and si.on_wait and len(si.on_wait) > 1:
        waits = list(si.on_wait)
        si.on_wait = waits[:1]
        for w in waits[1:]:
            d2 = self.nc.sync.drain()
            s2 = d2.ins.sync_info
            if s2 is None:
                d2.ins.sync_info = type(si)(on_wait=[w], on_update=[])
            else:
                s2.on_wait = [w]
    self.nc.all_engine_barrier()
    assert self.sems is not None
    popped = self.nc._tile_sem_poison_stack.pop()
    assert popped is self._sem_poison
    self.nc.clear_and_free_semaphores(list(self.sems.allocated().values()))
    self.nc.all_engine_barrier()

_TC._drain_and_barrier = _split_drain_and_barrier

B, H, DH, HID, LK = 16, 16, 64, 1024, 4096
NCORES = 8
BL = B // NCORES            # batches per core
NCH = HID // 128            # 8 hidden chunks of 128
NT = LK // 1024             # 4 stream tiles of 1024 seq positions per batch
NU = BL * NT                # 8 stream units per core
N_WARM = 8                  # PE warmup matmuls (N=512, ~0.43us each cold)

f32 = mybir.dt.float32
bf16 = mybir.dt.bfloat16
fp8 = mybir.dt.float8e3
FT = mybir.ActivationFunctionType
AX = mybir.AxisListType

bf16_np = ml_dtypes.bfloat16
fp8_np = ml_dtypes.float8_e3m4


def build_nc():
    nc = bass.Bass("TRN2")

    kt_d = nc.dram_tensor("kt_loc", [NU, 128, NCH, 1024], fp8, kind="ExternalInput")
    v_d = nc.dram_tensor("v_loc", [NU, 128, NCH, 1024], fp8, kind="ExternalInput")
    blob_d = nc.dram_tensor("blob", [128, 260], bf16, kind="ExternalInput")
    w_d = nc.dram_tensor("w_loc", [BL, H, HID], bf16, kind="ExternalOutput")
    z_d = nc.dram_tensor("z_loc", [H, BL], f32, kind="ExternalOutput")

    with TileContext(nc) as tc:
        with tc.tile_pool(name="main", bufs=1) as mp, \
             tc.tile_pool(name="ps", bufs=1, space="PSUM") as pp:

            # ---- constants ----
            blob = mp.tile([128, 260], bf16, tag="blob")
            nc.scalar.dma_start(out=blob, in_=blob_d[:, :])
            r_sb = blob[:, 0:256].rearrange("p (ch bl h) -> p ch bl h", ch=NCH, bl=BL)

            # ---- long-lived accumulators ----
            psum_w = [pp.tile([H, HID], f32, tag=f"w{bl}", name=f"w{bl}")
                      for bl in range(BL)]
            # per-(unit, half) softmax normalizer partials via ACT accum_out
            Zs = mp.tile([H, 2 * NU], f32, tag="Zs")

            # ---- PE warmup: keep the PE busy during the initial DMA fill so
            # the HAM clock gate reaches 8/8 before real work arrives.  The
            # warmup groups use psum_w[0]'s bank and close before the real
            # accumulation group starts. ----
            warm_sb = mp.tile([128, H], bf16, tag="warm")
            nc.vector.memset(warm_sb, 0.0)
            warm_mv = mp.tile([128, 512], bf16, tag="warmmv")
            nc.vector.memset(warm_mv, 0.0)
            for i in range(N_WARM):
                nc.tensor.matmul(psum_w[0][:, 0:512], warm_sb, warm_mv,
                                 start=True, stop=True)

            # ---- fused stream: 8 units of (kt 1MB fp8, vt 2MB bf16).
            # The V phase runs one unit behind scores (software pipeline) so
            # the DVE transposes of unit u overlap with scores of unit u+1
            # and the PE never waits on DVE. ----
            def emit_v_phase(uu, attnT_u, vt_u):
                bl_u, t_u = divmod(uu, NT)
                nc.tensor.ldweights(vt_u[:, 0, 0:1])
                for j in range(8):
                    first = (t_u == 0 and j == 0)
                    last = (t_u == NT - 1 and j == 7)
                    for hf in range(2):
                        nc.tensor.matmul(
                            psum_w[bl_u][:, hf * 512:(hf + 1) * 512],
                            attnT_u[:, j, 0:H],
                            vt_u[:, j, hf * 512:(hf + 1) * 512],
                            start=first, stop=last)

            # All transfers queue upfront with no buffer reuse (no WAR
            # waits): kt on the SP HWDGE ring, vt via GpSimd SWDGE.  An HWDGE
            # dma_start blocks its issuing queue until the transfer completes,
            # so the streams live on queues with no compute work; ACT stays
            # free for the exps.  The SDMA engines round-robin the two queues
            # per descriptor (vt descriptors are 2x kt's bytes), splitting
            # bandwidth 2:1 vt:kt — the exact consumption ratio, so kt(u) and
            # vt(u) arrive just in time at a ~8.8us unit cadence.
            kts, vts = [], []
            for u in range(NU):
                kt = mp.tile([128, NCH, 1024], fp8, tag="kt", bufs=NU,
                             name=f"kt{u}")
                nc.sync.dma_start(out=kt, in_=kt_d[u])
                kts.append(kt)
            for u in range(NU):
                vt = mp.tile([128, NCH, 1024], fp8, tag="vt", bufs=NU,
                             name=f"vt{u}")
                nc.gpsimd.dma_start(out=vt, in_=v_d[u])
                vts.append(vt)

            scratch2 = mp.tile([1, NU], f32, tag="scratch2")
            attnTs = []
            prev = None
            for u in range(NU):
                bl, t = divmod(u, NT)
                kt = kts[u]
                vt = vts[u]

                # scores[h, s] = sum_c r[c, h] kt[c, s]: r stationary, kt
                # moving in wide N=512 matmuls.  The leading ldweights absorbs
                # the kt-DMA wait.
                nc.tensor.ldweights(kt[:, 0, 0:1])
                attn = mp.tile([32, 8, 4, 32], bf16, tag="attn", bufs=2)
                attnT = mp.tile([128, 8, 32], bf16, tag="attnT", bufs=NU)
                attn_flat = attn.rearrange("p j b c -> p (j b c)")
                # attn bufs=2: the buffer WAR vs the DVE transposes of unit
                # u-2 is taught to the ACT clock by a ratchet copy reading
                # attnT(u-1).
                rT = None
                if u >= 2:
                    rT = nc.scalar.copy(scratch2[0:1, u - 2:u - 1],
                                        attnTs[u - 1][0:1, 0, 0:1])
                for h in range(2):
                    pss = pp.tile([H, 512], f32, tag="s", bufs=4)
                    for cj in range(NCH):
                        nc.tensor.matmul(
                            pss,
                            r_sb[:, cj, bl, :],
                            kt[:, cj, h * 512:(h + 1) * 512],
                            start=(cj == 0), stop=(cj == NCH - 1))
                    ei = nc.scalar.activation(
                        attn_flat[0:H, h * 512:(h + 1) * 512], pss, FT.Exp,
                        accum_out=Zs[:, 2 * u + h:2 * u + h + 1])
                    if rT is not None and h == 0:
                        add_dep_helper(ei.ins, rT.ins, reason="act-order")
                    # DVE assembles attnT via batched 32x32 block transposes:
                    # one instruction per 32-partition destination group moves
                    # 4 blocks (rows 16:32 of attn are junk; the junk columns
                    # of attnT are never read).
                    for b in range(4):
                        nc.vector.transpose(
                            attnT[b * 32:(b + 1) * 32, 4 * h:4 * h + 4, :],
                            attn[0:32, 4 * h:4 * h + 4, b, :])

                attnTs.append(attnT)
                if prev is not None:
                    emit_v_phase(u - 1, *prev)
                prev = (attnT, vt)

            emit_v_phase(NU - 1, *prev)

            # ---- drain: PSUM -> SBUF -> DRAM ----
            w_sb = mp.tile([H, BL, HID], bf16, tag="wsb")
            z_sb = mp.tile([H, BL], f32, tag="zsb")
            for bl in range(BL):
                nc.vector.tensor_copy(w_sb[:, bl, :], psum_w[bl])
            for bl in range(BL):
                nc.vector.reduce_sum(z_sb[:, bl:bl + 1],
                                     Zs[:, bl * 2 * NT:(bl + 1) * 2 * NT],
                                     axis=AX.X)
            # ratchets: teach the ACT clock the final DVE ticks (the bl=1
            # drain copies are the last DVE work) so the output DMAs carry
            # only their ring wait; explicit deps pin the order.
            scratch = mp.tile([1, 2], f32, tag="scratch")
            ratA = nc.scalar.copy(scratch[0:1, 0:1], w_sb[0:1, 1, 0:1])
            ratB = nc.scalar.copy(scratch[0:1, 1:2], z_sb[0:1, 1:2])
            add_dep_helper(ratB.ins, ratA.ins, reason="act-order")
            for bl in range(BL):
                d = nc.scalar.dma_start(out=w_d[bl], in_=w_sb[:, bl, :])
                add_dep_helper(d.ins, ratB.ins, reason="act-order")
            d = nc.scalar.dma_start(out=z_d[:, :], in_=z_sb)
            add_dep_helper(d.ins, ratB.ins, reason="act-order")

    # Post-pass: strip redundant semaphore waits so no instruction exceeds
    # its struct's 1-wait budget.  Every engine sequencer issues in FIFO
    # order, so (a) a wait on the engine's own completion semaphore is
    # trivially satisfied, and (b) a wait on sem>=v is redundant when an
    # earlier instruction on the same engine already waited sem>=v'>=v.
    eng_sem = {
        mybir.EngineType.Activation: "Activation",
        mybir.EngineType.PE: "PE",
        mybir.EngineType.DVE: "DVE",
        mybir.EngineType.SP: "SP",
        mybir.EngineType.Pool: "Pool",
    }
    known = {}
    for fn in nc.m.functions:
        for b in fn.blocks:
            for ins in b.instructions:
                si = ins.sync_info
                if si is None or not si.on_wait:
                    continue
                eng = ins.engine
                kn = known.setdefault(eng, {})
                if len(si.on_wait) > 1:
                    own = eng_sem.get(eng)
                    keep = []
                    for w in si.on_wait:
                        if own is not None and w.ant_name.startswith(own + "_"):
                            continue
                        if kn.get(w.ant_name, -1) >= w.wait_value:
                            continue
                        keep.append(w)
                    if not keep:
                        keep = list(si.on_wait)[:1]
                    si.on_wait = keep
                for w in si.on_wait:
                    if w.wait_value > kn.get(w.ant_name, -1):
                        kn[w.ant_name] = w.wait_value

    return nc


def _quant_fb_e3m4(v):
    """Quantize [BL, LK, HID] to e3m4 with error feedback along seq."""
    v = np.ascontiguousarray(v, np.float32)
    vq = np.empty(v.shape, dtype=fp8_np)
    carry = np.zeros((v.shape[0], v.shape[2]), np.float32)
    for s in range(v.shape[1]):
        x = v[:, s, :] + carry
        xq = x.astype(fp8_np)
        carry = x - xq.astype(np.float32)
        vq[:, s, :] = xq
    return vq.astype(np.float32)


def make_in_maps(q, k, v, Wq, bq, Wv, bv, Wo, bo, Wk):
    scale = DH ** -0.5
    q64, Wq64, bq64, Wk64 = (np.asarray(a, np.float64) for a in (q, Wq, bq, Wk))
    qh = q64 @ Wq64 + bq64                      # [B, HID]
    r = np.zeros((B, H, HID))
    for h in range(H):
        blk = slice(h * DH, (h + 1) * DH)
        r[:, h, :] = qh[:, blk] @ Wk64[:, blk].T * scale

    in_maps = []
    for c in range(NCORES):
        sl = slice(BL * c, BL * (c + 1))
        # kt: [u, p, ch, s] = k[bl, t*1024+s, ch*128+p], u = bl*NT + t
        kk = np.ascontiguousarray(k[sl].transpose(0, 2, 1))        # [BL, HID, LK]
        kt = kk.reshape(BL, NCH, 128, NT, 1024).transpose(0, 3, 2, 1, 4) \
            .reshape(NU, 128, NCH, 1024)
        # vt: [u, p, blk, c] = vq[bl, t*1024 + blk*128 + p, c] where vq is v
        # quantized to e3m4 with error feedback along the seq axis (attention
        # averages over seq with near-uniform weights, so noise-shaped
        # quantization error largely cancels in attn @ v).
        vq = _quant_fb_e3m4(v[sl])
        vt = vq.reshape(BL, NT, 8, 128, HID).transpose(0, 1, 3, 2, 4) \
            .reshape(NU, 128, 8, 1024)
        blob = np.zeros((128, 260), dtype=np.float64)
        rc = r[sl]                                                  # [BL, H, HID]
        # blob[p, (ch, bl, h)] = rc[bl, h, ch*128+p]
        blob[:, 0:256] = rc.transpose(2, 0, 1).reshape(NCH, 128, BL, H) \
            .transpose(1, 0, 2, 3).reshape(128, 256)
        blob[:, 256] = 1.0
        in_maps.append({
            "kt_loc": np.ascontiguousarray(kt).astype(fp8_np),
            "v_loc": np.ascontiguousarray(vt).astype(fp8_np),
            "blob": blob.astype(bf16_np),
        })
    return in_maps


def finish_host(results, Wv, bv, Wo, bo):
    """Apply the Wv / Wo tail on the host: out = concat_h((w/Z) Wv_h + bv) Wo + bo."""
    Wv64, bv64 = np.asarray(Wv, np.float64), np.asarray(bv, np.float64)
    Wo64, bo64 = np.asarray(Wo, np.float64), np.asarray(bo, np.float64)
    out = np.zeros((B, HID))
    for c, res in enumerate(results):
        w = np.asarray(res["w_loc"], np.float64)       # [BL, H, HID]
        z = np.asarray(res["z_loc"], np.float64)       # [H, BL]
        for bl in range(BL):
            x = np.zeros(HID)
            xs = []
            for h in range(H):
                blk = slice(h * DH, (h + 1) * DH)
                xs.append(w[bl, h] / z[h, bl] @ Wv64[:, blk] + bv64[blk])
            x = np.concatenate(xs)
            out[c * BL + bl] = x @ Wo64 + bo64
    return out.astype(np.float32)


_nc_cache = None


def kernel(q, k, v, index_sample, Wq, bq, Wk, bk, Wv, bv, Wo, bo):
    global _nc_cache
    q, k, v = (np.asarray(a, np.float32) for a in (q, k, v))
    # bk provably cancels in the softmax (constant shift per (b, h) row).
    if _nc_cache is None:
        _nc_cache = build_nc()
    nc = _nc_cache
    in_maps = make_in_maps(q, k, v, Wq, bq, Wv, bv, Wo, bo, Wk)
    res = bass_utils.run_bass_kernel_spmd(nc, in_maps, core_ids=list(range(NCORES)))
    return finish_host(res.results, Wv, bv, Wo, bo)


# revision 49
# speedup vs baseline: 2.0315x; 1.1589x over previous
"""Trainium2 Bass kernel for ProbSparse multi-head attention (L_Q = 1).

Math: with L_Q=1 the reference's top-k/sampling machinery is identity
(top-1 of a length-1 axis is index 0 and the scatter overwrites the whole
context), so the computation reduces to single-query attention:

  out[b] = concat_h( softmax((q Wq)_h . (k Wk)_h^T / 8) @ (v Wv)_h ) @ Wo + bo

Low-rank restructuring (L_Q = 1): fold the projections through the single
query / attention vector so the big k/v tensors are consumed by exactly one
streaming pass each:

  r[b,h,:]   = Wk_h @ (qh[b,h]/8)          (host, f64)
  scoresT[s,h] = sum_c k[b,s,c] r[b,h,c]   (device, PE)
  attnT      = exp(scoresT)                 (device, ACT; unnormalized)
  w[b,h,:]   = sum_s attnT[s,h] v[b,s,:]    (device, PE)
  Z[b,h]     = sum_s attnT[s,h]             (device, PE; ones column)
  out[b]     = concat_h((w/Z) Wv_h + bv_h) @ Wo + bo   (host)

v3 (this file): everything except the two streaming passes moves to the
host.  k is staged as fp8 e3m4 (4-bit mantissa; measured end-to-end error
0.0066 vs the 2e-2 gate), v as bf16.  Scores use kt as the matmul
STATIONARY operand so PSUM receives scoresT [128s, 16h] directly: no PE
transposes, full-width ACT exp, and attnT feeds the V-phase matmuls as
stationary with vt moving.  Per-core DMA is 24.3 MB vs the v2 design's
38 MB, and the single fused stream keeps DMA busy continuously while PE
gaps stay under the HAM re-throttle window.

Sharding: data-parallel over batch, 2 batches per core, 8 cores.
"""

import sys

sys.path.insert(0, "/opt/trn_rl_repo")

import numpy as np
import ml_dtypes

import concourse.bass as bass
import concourse.mybir as mybir
from bass_rust import add_dep_helper
import concourse.tile_sem_assignment as _tsa
from concourse.tile import TileContext
from concourse import bass_utils


# ---- framework patch (from v2): the kernel-tail drain aggregates one
# semaphore wait per active proc, exceeding the 1-wait DRAIN encoding.
# Split the waits across a chain of single-wait drains.
from concourse.tile import TileContext as _TC
from concourse.vector_clock import ScopedClock as _SC

def _split_drain_and_barrier(self, tick_clock, wait_clock):
    drain_inst = self.nc.sync.drain()
    wait_clock.add_sem_waits(drain_inst.ins, _SC({None: tick_clock.global_clock}))
    si = drain_inst.ins.sync_info
    if si is not None and si.on_wait and len(si.on_wait) > 1:
        waits = list(si.on_wait)
        si.on_wait = waits[:1]
        for w in waits[1:]:
            d2 = self.nc.sync.drain()
            s2 = d2.ins.sync_info
            if s2 is None:
                d2.ins.sync_info = type(si)(on_wait=[w], on_update=[])
            else:
                s2.on_wait = [w]
    self.nc.all_engine_barrier()
    assert self.sems is not None
    popped = self.nc._tile_sem_poison_stack.pop()
    assert popped is self._sem_poison
    self.nc.clear_and_free_semaphores(list(self.sems.allocated().values()))
    self.nc.all_engine_barrier()

_TC._drain_and_barrier = _split_drain_and_barrier

B, H, DH, HID, LK = 16, 16, 64, 1024, 4096
NCORES = 8
BL = B // NCORES            # batches per core
NCH = HID // 128            # 8 hidden chunks of 128
NT = LK // 1024             # 4 stream tiles of 1024 seq positions per batch
NU = BL * NT                # 8 stream units per core
N_WARM = 12                 # PE warmup matmuls (N=512, ~0.43us each cold)

f32 = mybir.dt.float32
bf16 = mybir.dt.bfloat16
fp8 = mybir.dt.float8e3
FT = mybir.ActivationFunctionType
AX = mybir.AxisListType

bf16_np = ml_dtypes.bfloat16
fp8_np = ml_dtypes.float8_e3m4


def build_nc():
    nc = bass.Bass("TRN2")

    kt_d = nc.dram_tensor("kt_loc", [NU, 128, NCH, 1024], fp8, kind="ExternalInput")
    v_d = nc.dram_tensor("v_loc", [NU, 128, NCH, 1024], fp8, kind="ExternalInput")
    blob_d = nc.dram_tensor("blob", [128, 260], bf16, kind="ExternalInput")
    w_d = nc.dram_tensor("w_loc", [BL, H, HID], bf16, kind="ExternalOutput")
    z_d = nc.dram_tensor("z_loc", [H, BL], f32, kind="ExternalOutput")

    with TileContext(nc) as tc:
        with tc.tile_pool(name="main", bufs=1) as mp, \
             tc.tile_pool(name="ps", bufs=1, space="PSUM") as pp:

            # ---- constants ----
            blob = mp.tile([128, 260], bf16, tag="blob")
            nc.scalar.dma_start(out=blob, in_=blob_d[:, :])
            r_sb = blob[:, 0:256].rearrange("p (ch bl h) -> p ch bl h", ch=NCH, bl=BL)

            # ---- long-lived accumulators ----
            # Each batch's w accumulates in two PE column-group partitions
            # (rows 0:16 and 32:48) so V matmuls for adjacent j run
            # concurrently in separate column groups; one DVE add merges them
            # at drain time.
            psum_w = [pp.tile([48, HID], f32, tag=f"w{bl}", name=f"w{bl}")
                      for bl in range(BL)]
            # per-(unit, half) softmax normalizer partials via ACT accum_out
            Zs = mp.tile([H, 2 * NU], f32, tag="Zs")

            # ---- PE warmup: keep the PE busy during the initial DMA fill so
            # the HAM clock gate reaches 8/8 before real work arrives.  The
            # warmup groups use psum_w[0]'s bank and close before the real
            # accumulation group starts. ----
            warm_sb = mp.tile([128, H], bf16, tag="warm")
            nc.vector.memset(warm_sb, 0.0)
            warm_mv = mp.tile([128, 512], bf16, tag="warmmv")
            nc.vector.memset(warm_mv, 0.0)
            for i in range(N_WARM):
                nc.tensor.matmul(psum_w[0][0:H, 0:512], warm_sb, warm_mv,
                                 start=True, stop=True)

            # ---- fused stream: 8 units of (kt 1MB fp8, vt 2MB bf16).
            # The V phase runs one unit behind scores (software pipeline) so
            # the DVE transposes of unit u overlap with scores of unit u+1
            # and the PE never waits on DVE. ----
            def emit_v_phase(uu, attnT_u, vt_u):
                bl_u, t_u = divmod(uu, NT)
                nc.tensor.ldweights(vt_u[:, 0, 0:1])
                for j in range(8):
                    g = j % 2
                    first = (t_u == 0 and j == g)
                    last = (t_u == NT - 1 and j == 6 + g)
                    for hf in range(2):
                        nc.tensor.matmul(
                            psum_w[bl_u][32 * g:32 * g + H,
                                         hf * 512:(hf + 1) * 512],
                            attnT_u[:, j, 0:H],
                            vt_u[:, j, hf * 512:(hf + 1) * 512],
                            start=first, stop=last,
                            tile_position=(0, 32 * g))

            # All transfers queue upfront with no buffer reuse (no WAR
            # waits): kt on the SP HWDGE ring, vt via GpSimd SWDGE.  An HWDGE
            # dma_start blocks its issuing queue until the transfer completes,
            # so the streams live on queues with no compute work; ACT stays
            # free for the exps.  The SDMA engines round-robin the two queues
            # per descriptor (vt descriptors are 2x kt's bytes), splitting
            # bandwidth 2:1 vt:kt — the exact consumption ratio, so kt(u) and
            # vt(u) arrive just in time at a ~8.8us unit cadence.
            kts, vts = [], []
            for u in range(NU):
                kt = mp.tile([128, NCH, 1024], fp8, tag="kt", bufs=NU,
                             name=f"kt{u}")
                nc.sync.dma_start(out=kt, in_=kt_d[u])
                kts.append(kt)
            for u in range(NU):
                vt = mp.tile([128, NCH, 1024], fp8, tag="vt", bufs=NU,
                             name=f"vt{u}")
                nc.gpsimd.dma_start(out=vt, in_=v_d[u])
                vts.append(vt)

            scratch2 = mp.tile([1, NU], f32, tag="scratch2")
            attnTs = []
            prev = None
            for u in range(NU):
                bl, t = divmod(u, NT)
                kt = kts[u]
                vt = vts[u]

                # scores[h, s] = sum_c r[c, h] kt[c, s]: r stationary, kt
                # moving in wide N=512 matmuls.  The leading ldweights absorbs
                # the kt-DMA wait.
                nc.tensor.ldweights(kt[:, 0, 0:1])
                attn = mp.tile([32, 8, 4, 32], bf16, tag="attn", bufs=2)
                attnT = mp.tile([128, 8, 32], bf16, tag="attnT", bufs=NU)
                attn_flat = attn.rearrange("p j b c -> p (j b c)")
                # attn bufs=2: the buffer WAR vs the DVE transposes of unit
                # u-2 is taught to the ACT clock by a ratchet copy reading
                # attnT(u-1).
                rT = None
                if u >= 2:
                    rT = nc.scalar.copy(scratch2[0:1, u - 2:u - 1],
                                        attnTs[u - 1][0:1, 0, 0:1])
                # The two 512-column halves run as concurrent accumulation
                # chains in separate PE column groups (rows 0:16 / 32:48 of
                # one psum tile).
                pss2 = pp.tile([48, 512], f32, tag="s", bufs=4)
                for cj in range(NCH):
                    for h in range(2):
                        nc.tensor.matmul(
                            pss2[32 * h:32 * h + H, :],
                            r_sb[:, cj, bl, :],
                            kt[:, cj, h * 512:(h + 1) * 512],
                            start=(cj == 0), stop=(cj == NCH - 1),
                            tile_position=(0, 32 * h))
                for h in range(2):
                    ei = nc.scalar.activation(
                        attn_flat[0:H, h * 512:(h + 1) * 512],
                        pss2[32 * h:32 * h + H, :], FT.Exp,
                        accum_out=Zs[:, 2 * u + h:2 * u + h + 1])
                    if rT is not None and h == 0:
                        add_dep_helper(ei.ins, rT.ins, reason="act-order")
                    # DVE assembles attnT via batched 32x32 block transposes:
                    # one instruction per 32-partition destination group moves
                    # 4 blocks (rows 16:32 of attn are junk; the junk columns
                    # of attnT are never read).
                    for b in range(4):
                        nc.vector.transpose(
                            attnT[b * 32:(b + 1) * 32, 4 * h:4 * h + 4, :],
                            attn[0:32, 4 * h:4 * h + 4, b, :])

                attnTs.append(attnT)
                if prev is not None:
                    emit_v_phase(u - 1, *prev)
                prev = (attnT, vt)

            emit_v_phase(NU - 1, *prev)

            # ---- drain: PSUM -> SBUF -> DRAM ----
            w_sb = mp.tile([H, BL, HID], bf16, tag="wsb")
            wtmp = mp.tile([H, BL, HID], f32, tag="wtmp")
            z_sb = mp.tile([H, BL], f32, tag="zsb")
            for bl in range(BL):
                nc.vector.tensor_copy(wtmp[:, bl, :], psum_w[bl][0:H, :])
                nc.vector.tensor_add(w_sb[:, bl, :], wtmp[:, bl, :],
                                     psum_w[bl][32:32 + H, :])
            for bl in range(BL):
                nc.vector.reduce_sum(z_sb[:, bl:bl + 1],
                                     Zs[:, bl * 2 * NT:(bl + 1) * 2 * NT],
                                     axis=AX.X)
            # ratchets: teach the ACT clock the final DVE ticks (the bl=1
            # drain copies are the last DVE work) so the output DMAs carry
            # only their ring wait; explicit deps pin the order.
            scratch = mp.tile([1, 2], f32, tag="scratch")
            ratA = nc.scalar.copy(scratch[0:1, 0:1], w_sb[0:1, 1, 0:1])
            ratB = nc.scalar.copy(scratch[0:1, 1:2], z_sb[0:1, 1:2])
            add_dep_helper(ratB.ins, ratA.ins, reason="act-order")
            for bl in range(BL):
                d = nc.scalar.dma_start(out=w_d[bl], in_=w_sb[:, bl, :])
                add_dep_helper(d.ins, ratB.ins, reason="act-order")
            d = nc.scalar.dma_start(out=z_d[:, :], in_=z_sb)
            add_dep_helper(d.ins, ratB.ins, reason="act-order")

    # Post-pass: strip redundant semaphore waits so no instruction exceeds
    # its struct's 1-wait budget.  Every engine sequencer issues in FIFO
    # order, so (a) a wait on the engine's own completion semaphore is
    # trivially satisfied, and (b) a wait on sem>=v is redundant when an
    # earlier instruction on the same engine already waited sem>=v'>=v.
    eng_sem = {
        mybir.EngineType.Activation: "Activation",
        mybir.EngineType.PE: "PE",
        mybir.EngineType.DVE: "DVE",
        mybir.EngineType.SP: "SP",
        mybir.EngineType.Pool: "Pool",
    }
    known = {}
    for fn in nc.m.functions:
        for b in fn.blocks:
            for ins in b.instructions:
                si = ins.sync_info
                if si is None or not si.on_wait:
                    continue
                eng = ins.engine
                kn = known.setdefault(eng, {})
                if len(si.on_wait) > 1:
                    own = eng_sem.get(eng)
                    keep = []
                    for w in si.on_wait:
                        if own is not None and w.ant_name.startswith(own + "_"):
                            continue
                        if kn.get(w.ant_name, -1) >= w.wait_value:
                            continue
                        keep.append(w)
                    if not keep:
                        keep = list(si.on_wait)[:1]
                    si.on_wait = keep
                for w in si.on_wait:
                    if w.wait_value > kn.get(w.ant_name, -1):
                        kn[w.ant_name] = w.wait_value

    return nc


def _quant_fb_e3m4(v):
    """Quantize [BL, LK, HID] to e3m4 with error feedback along seq."""
    v = np.ascontiguousarray(v, np.float32)
    vq = np.empty(v.shape, dtype=fp8_np)
    carry = np.zeros((v.shape[0], v.shape[2]), np.float32)
    for s in range(v.shape[1]):
        x = v[:, s, :] + carry
        xq = x.astype(fp8_np)
        carry = x - xq.astype(np.float32)
        vq[:, s, :] = xq
    return vq.astype(np.float32)


def make_in_maps(q, k, v, Wq, bq, Wv, bv, Wo, bo, Wk):
    scale = DH ** -0.5
    q64, Wq64, bq64, Wk64 = (np.asarray(a, np.float64) for a in (q, Wq, bq, Wk))
    qh = q64 @ Wq64 + bq64                      # [B, HID]
    r = np.zeros((B, H, HID))
    for h in range(H):
        blk = slice(h * DH, (h + 1) * DH)
        r[:, h, :] = qh[:, blk] @ Wk64[:, blk].T * scale

    in_maps = []
    for c in range(NCORES):
        sl = slice(BL * c, BL * (c + 1))
        # kt: [u, p, ch, s] = k[bl, t*1024+s, ch*128+p], u = bl*NT + t
        kk = np.ascontiguousarray(k[sl].transpose(0, 2, 1))        # [BL, HID, LK]
        kt = kk.reshape(BL, NCH, 128, NT, 1024).transpose(0, 3, 2, 1, 4) \
            .reshape(NU, 128, NCH, 1024)
        # vt: [u, p, blk, c] = vq[bl, t*1024 + blk*128 + p, c] where vq is v
        # quantized to e3m4 with error feedback along the seq axis (attention
        # averages over seq with near-uniform weights, so noise-shaped
        # quantization error largely cancels in attn @ v).
        vq = _quant_fb_e3m4(v[sl])
        vt = vq.reshape(BL, NT, 8, 128, HID).transpose(0, 1, 3, 2, 4) \
            .reshape(NU, 128, 8, 1024)
        blob = np.zeros((128, 260), dtype=np.float64)
        rc = r[sl]                                                  # [BL, H, HID]
        # blob[p, (ch, bl, h)] = rc[bl, h, ch*128+p]
        blob[:, 0:256] = rc.transpose(2, 0, 1).reshape(NCH, 128, BL, H) \
            .transpose(1, 0, 2, 3).reshape(128, 256)
        blob[:, 256] = 1.0
        in_maps.append({
            "kt_loc": np.ascontiguousarray(kt).astype(fp8_np),
            "v_loc": np.ascontiguousarray(vt).astype(fp8_np),
            "blob": blob.astype(bf16_np),
        })
    return in_maps


def finish_host(results, Wv, bv, Wo, bo):
    """Apply the Wv / Wo tail on the host: out = concat_h((w/Z) Wv_h + bv) Wo + bo."""
    Wv64, bv64 = np.asarray(Wv, np.float64), np.asarray(bv, np.float64)
    Wo64, bo64 = np.asarray(Wo, np.float64), np.asarray(bo, np.float64)
    out = np.zeros((B, HID))
    for c, res in enumerate(results):
        w = np.asarray(res["w_loc"], np.float64)       # [BL, H, HID]
        z = np.asarray(res["z_loc"], np.float64)       # [H, BL]
        for bl in range(BL):
            x = np.zeros(HID)
            xs = []
            for h in range(H):
                blk = slice(h * DH, (h + 1) * DH)
                xs.append(w[bl, h] / z[h, bl] @ Wv64[:, blk] + bv64[blk])
            x = np.concatenate(xs)
            out[c * BL + bl] = x @ Wo64 + bo64
    return out.astype(np.float32)


_nc_cache = None


def kernel(q, k, v, index_sample, Wq, bq, Wk, bk, Wv, bv, Wo, bo):
    global _nc_cache
    q, k, v = (np.asarray(a, np.float32) for a in (q, k, v))
    # bk provably cancels in the softmax (constant shift per (b, h) row).
    if _nc_cache is None:
        _nc_cache = build_nc()
    nc = _nc_cache
    in_maps = make_in_maps(q, k, v, Wq, bq, Wv, bv, Wo, bo, Wk)
    res = bass_utils.run_bass_kernel_spmd(nc, in_maps, core_ids=list(range(NCORES)))
    return finish_host(res.results, Wv, bv, Wo, bo)
